# revision 1
# baseline (speedup 1.0000x reference)
"""Trainium2 Bass kernel for a 5-layer GPT-style transformer (BigramLanguageModel).

Sharding: data-parallel over batch (B=8 -> 1 sequence per core) through the
transformer layers (zero collectives), then a bf16 AllGather of the final
hidden states and a vocab-parallel LM head (each core computes all 4096
tokens x its 4000-wide vocab shard), with bf16 logits written to DRAM and
widened to f32 on the host.

Differences from the f32r baseline (598.7us):
  - all matmul operands in bf16 (weights pre-cast on host); residual stream,
    PSUM accumulation and LN statistics stay f32.  Kills the f32r <256-row
    4x penalty and halves weight/activation DMA.
  - causal mask applied by an extra PE matmul accumulated into the scores
    PSUM (rank-128 factorization of the triangular mask), not a DVE add.
  - attention output computed in natural [t, head] orientation with an
    appended ones-column in V producing the softmax denominator as a
    per-partition scalar (reciprocal + tensor_scalar; no broadcast matmul).
  - FF2 runs m-major (all 12 FF1 outputs kept in SBUF) so each token tile's
    residual add + next LN chain starts while the remaining tiles' FF2
    matmuls keep the PE busy; same per-tile fusion after proj.
  - next layer's weights prefetched at layer start (double-buffered pools,
    SWDGE queue) so the PE never waits on weight DMA.
  - logits staged and written as bf16 (the head is otherwise DMA-bound).
  - one ACT function table (natural_log_exp_and_others) preloaded up front
    and rstd computed as exp(-0.5*ln(var+eps)): Sqrt and Exp can never share
    a table, so the Sqrt-based LN forced two 1.3us table reloads per layer.
"""

import sys

import numpy as np

sys.path.insert(0, "/opt/trn_rl_repo")

import ml_dtypes

import concourse.bass as bass
import concourse.mybir as mybir
import concourse.tile as tile
from concourse import bacc
from concourse.bass_utils import run_bass_kernel_spmd

F32 = mybir.dt.float32
BF = mybir.dt.bfloat16
I32 = mybir.dt.int32
AF = mybir.ActivationFunctionType
ALU = mybir.AluOpType
BF_NP = ml_dtypes.bfloat16

D, H, HS, L, V, CTX, B, T, FF = 384, 6, 64, 5, 32000, 512, 8, 512, 1536
P = 128
DT = D // P          # 3 d-tiles
TT = T // P          # 4 t-tiles
NT = FF // P         # 12 ff-tiles
N_CORES = 8
VSH = V // N_CORES   # 4000 vocab shard per core
VCH = 500            # vocab chunk per matmul (PSUM bank = 512 f32)
NCH = VSH // VCH     # 8 chunks per 128-token row
SCALE = float(D) ** -0.5
EPS = 1e-5
MASKC = 8388608.0    # -MASKC*(s-t) added to masked scores; exact in bf16
POOL_LN = False      # LN statistics via gpsimd scalar_tensor_tensor accumulate


def _build(trivial_gb=False, trivial_bias=False, trivial_bout=False,
           sim_nocc=False):
    nc = bacc.Bacc("TRN2", target_bir_lowering=False, debug=False,
                   num_devices=1 if sim_nocc else N_CORES)

    io = {}
    io["x"] = nc.dram_tensor("x", [T], I32, kind="ExternalInput")
    io["tok_emb"] = nc.dram_tensor("tok_emb", [V, D], F32, kind="ExternalInput")
    io["pos_emb"] = nc.dram_tensor("pos_emb", [CTX, D], F32, kind="ExternalInput")
    for n, shp in [("ln1_g", [L, D]), ("ln1_b", [L, D]),
                   ("bproj", [L, D]), ("ln2_g", [L, D]), ("ln2_b", [L, D]),
                   ("b1", [L, FF]), ("b2", [L, D]),
                   ("lnf_g", [D]), ("lnf_b", [D]), ("bout_sh", [VSH])]:
        io[n] = nc.dram_tensor(n, shp, F32, kind="ExternalInput")
    for n, shp in [("Wq", [L, D, D]), ("Wk", [L, D, D]), ("Wv", [L, D, D]),
                   ("Wproj", [L, D, D]), ("W1", [L, D, FF]), ("W2", [L, FF, D]),
                   ("Wout_sh", [D, VSH])]:
        io[n] = nc.dram_tensor(n, shp, BF, kind="ExternalInput")
    io["logits_sh"] = nc.dram_tensor("logits_sh", [B * T, VSH], BF,
                                     kind="ExternalOutput")

    io["ident_d"] = nc.inline_tensor(np.eye(P).astype(BF_NP), name="ident_c")
    a_np = np.triu(np.ones((P, P), np.float32), 1)            # A[p,s]=1 if p<s
    b_np = np.tril(np.ones((P, P), np.float32)) * (-MASKC)    # B[p,t]=-c if p>=t
    io["maskA_d"] = nc.inline_tensor(a_np.astype(BF_NP), name="maskA_c")
    io["maskB_d"] = nc.inline_tensor(b_np.astype(BF_NP), name="maskB_c")

    with tile.TileContext(nc) as tc:
        _emit(nc, tc, io, trivial_gb, trivial_bias, trivial_bout, sim_nocc)
    nc.compile()
    return nc


def _emit(nc, tc, io, trivial_gb, trivial_bias, trivial_bout, sim_nocc):
    from contextlib import ExitStack
    with ExitStack() as octx:
        dram = octx.enter_context(tc.tile_pool(name="dram", bufs=1, space="DRAM"))
        pp = octx.enter_context(tc.tile_pool(name="pp", bufs=1, space="PSUM"))
        sb = octx.enter_context(tc.tile_pool(name="sb", bufs=1))

        def ps_tile(name, w=T, dt=F32):
            # shared 6-bank PSUM ring: transposes/qk/v/scores/ff1/proj/head
            return pp.tile([P, w], dt, name=name, tag="ps", bufs=6)

        def acc_tile(name, shape):
            # 2-bank PSUM ring: AV accumulators + FF2 accumulators
            return pp.tile(shape, F32, name=name, tag="accA", bufs=2)

        # ---- startup: embedding critical path first, weights later ----
        h_sb, idx_t = [], []
        for m in range(TT):
            it = sb.tile([P, 1], I32, name=f"idx{m}", tag="idx", bufs=4)
            nc.sync.dma_start(out=it[:], in_=io["x"][P * m:P * (m + 1), None])
            idx_t.append(it)
        pos_t = {}
        for m in range(2):
            pos_t[m] = sb.tile([P, D], F32, name=f"pos{m}", tag="pf", bufs=2)
            nc.sync.dma_start(out=pos_t[m][:],
                              in_=io["pos_emb"][P * m:P * (m + 1), :])
        ident = sb.tile([P, P], BF, name="ident", tag="cI", bufs=1)
        nc.sync.dma_start(out=ident[:], in_=io["ident_d"][:])
        for m in range(TT):
            ht = sb.tile([P, D], F32, name=f"h{m}", tag=f"h{m}", bufs=1)
            nc.gpsimd.indirect_dma_start(
                out=ht[:], out_offset=None, in_=io["tok_emb"][:],
                in_offset=bass.IndirectOffsetOnAxis(ap=idx_t[m][:, :1], axis=0))
            h_sb.append(ht)
        maskA = sb.tile([P, P], BF, name="maskA", tag="cA", bufs=1)
        nc.sync.dma_start(out=maskA[:], in_=io["maskA_d"][:])
        maskB = sb.tile([P, P], BF, name="maskB", tag="cB", bufs=1)
        nc.sync.dma_start(out=maskB[:], in_=io["maskB_d"][:])
        epsP = sb.tile([P, 1], F32, name="epsP", tag="cE", bufs=1)
        nc.vector.memset(epsP[:], EPS)

        # preload the one ACT function table that covers everything we use
        # (Exp+Ln+Relu+Copy+Identity); without this the table-load pass
        # thrashes between the exp and ln tables on every LN chain
        from concourse.hw_specs import get_activation_tables
        tset = list(get_activation_tables(nc.m.arch)).index(
            "natural_log_exp_and_others")
        nc.scalar.add_instruction(mybir.InstLoadActFuncSet(
            name=nc.get_next_instruction_name(), ins=[], outs=[],
            act_func_set_id=tset))

        # ---- weight loads (gpsimd SWDGE; layer 0+1 now, rest prefetched) ----
        wq_t, wk_t, wv_t, wpj_t, w1_t, w2_t, b1_t = ({} for _ in range(7))

        def load_weights(l):
            for nm, dr, store, kshape in (
                    ("wq", io["Wq"], wq_t, (DT, D)), ("wk", io["Wk"], wk_t, (DT, D)),
                    ("wv", io["Wv"], wv_t, (DT, D)),
                    ("wp", io["Wproj"], wpj_t, (DT, D)),
                    ("w1", io["W1"], w1_t, (DT, FF))):
                t = sb.tile([P, *kshape], BF, name=f"{nm}{l}", tag=nm, bufs=2)
                nc.sync.dma_start(
                    out=t[:], in_=dr[l].rearrange("(k p) n -> p k n", p=P))
                store[l] = t
            t = sb.tile([P, NT, D], BF, name=f"w2_{l}", tag="w2", bufs=2)
            nc.sync.dma_start(
                out=t[:], in_=io["W2"][l].rearrange("(k p) n -> p k n", p=P))
            w2_t[l] = t
            bt = sb.tile([P, NT], F32, name=f"b1c{l}", tag="b1c", bufs=2)
            nc.sync.dma_start(
                out=bt[:], in_=io["b1"][l, :].rearrange("(n p) -> p n", p=P))
            b1_t[l] = bt

        for m in range(TT):
            nc.vector.tensor_tensor(out=h_sb[m][:], in0=h_sb[m][:],
                                    in1=pos_t[m][:], op=ALU.add)
            if m + 2 < TT:
                pos_t[m + 2] = sb.tile([P, D], F32, name=f"pos{m+2}", tag="pf",
                                       bufs=2)
                nc.sync.dma_start(out=pos_t[m + 2][:],
                                  in_=io["pos_emb"][P * (m + 2):P * (m + 3), :])

        load_weights(0)
        wo = None

        # ---- LN helpers ----
        # chain spread over 4 engines so the per-tile latency pipeline is
        # short on each: DVE add -> Pool stats/aggr -> ACT sqrt -> DVE recip
        # -> Pool (-mean*rstd) -> ACT apply (Identity: x*rstd + (-mean*rstd))
        def ln_chain(m, g_dram, b_dram, tag, psum_add=None, bias_bc=None,
                     alt=False):
            """residual add (opt) + layernorm of h_sb[m] -> bf16 tile.
            alt=True runs the stats/apply on ACT so two chains can proceed
            in parallel (DVE chains serialize at phase boundaries)."""
            if psum_add is not None:
                if bias_bc is not None:
                    nc.vector.tensor_tensor(out=psum_add[:], in0=psum_add[:],
                                            in1=bias_bc[:], op=ALU.add)
                nc.vector.tensor_tensor(out=h_sb[m][:], in0=h_sb[m][:],
                                        in1=psum_add[:], op=ALU.add)
            if alt and trivial_gb:
                dmy = sb.tile([P, D], BF, name=f"dm_{tag}{m}", tag="dmy", bufs=2)
                s_t = sb.tile([P, 2], F32, name=f"s_{tag}{m}", tag="st2", bufs=4)
                nc.scalar.activation(out=dmy[:], in_=h_sb[m][:], func=AF.Copy,
                                     accum_out=s_t[:, 0:1])
                nc.scalar.activation(out=dmy[:], in_=h_sb[m][:], func=AF.Square,
                                     accum_out=s_t[:, 1:2])
                mv = sb.tile([P, 2], F32, name=f"mv_{tag}{m}", tag="mv", bufs=4)
                nc.vector.tensor_scalar(out=mv[:], in0=s_t[:], scalar1=1.0 / D,
                                        scalar2=None, op0=ALU.mult)
                nc.vector.scalar_tensor_tensor(
                    out=mv[:, 1:2], in0=mv[:, 0:1], scalar=mv[:, 0:1],
                    in1=mv[:, 1:2], op0=ALU.mult, op1=ALU.subtract)
                nc.scalar.activation(out=mv[:, 1:2], in_=mv[:, 1:2],
                                     func=AF.Sqrt, bias=epsP[:], scale=-1.0)
                nc.vector.reciprocal(out=mv[:, 1:2], in_=mv[:, 1:2])
                at = sb.tile([P, D], BF, name=f"a_{tag}{m}", tag="af", bufs=4)
                nmr = sb.tile([P, 1], F32, name=f"nm_{tag}{m}", tag="nmr", bufs=4)
                nc.gpsimd.tensor_scalar(out=nmr[:], in0=mv[:, 0:1],
                                        scalar1=mv[:, 1:2], scalar2=-1.0,
                                        op0=ALU.mult, op1=ALU.mult)
                nc.scalar.activation(out=at[:], in_=h_sb[m][:], func=AF.Identity,
                                     bias=nmr[:], scale=mv[:, 1:2])
                return at
            if POOL_LN:
                sq = sb.tile([P, D], F32, name=f"sq_{tag}{m}", tag="sq", bufs=2)
                s_t = sb.tile([P, 2], F32, name=f"s_{tag}{m}", tag="st2", bufs=4)
                nc.gpsimd.scalar_tensor_tensor(
                    out=sq[:], in0=h_sb[m][:], scalar=1.0, in1=h_sb[m][:],
                    op0=ALU.mult, op1=ALU.bypass, accum_out=s_t[:, 0:1])
                nc.gpsimd.scalar_tensor_tensor(
                    out=sq[:], in0=h_sb[m][:], scalar=1.0, in1=h_sb[m][:],
                    op0=ALU.mult, op1=ALU.mult, accum_out=s_t[:, 1:2])
                mv = sb.tile([P, 2], F32, name=f"mv_{tag}{m}", tag="mv", bufs=4)
                nc.gpsimd.tensor_scalar(out=mv[:], in0=s_t[:], scalar1=1.0 / D,
                                        scalar2=None, op0=ALU.mult)
                # mean^2 - meansq = -var; Sqrt(-1*in + eps) = sqrt(var+eps)
                nc.gpsimd.scalar_tensor_tensor(
                    out=mv[:, 1:2], in0=mv[:, 0:1], scalar=mv[:, 0:1],
                    in1=mv[:, 1:2], op0=ALU.mult, op1=ALU.subtract)
                nc.scalar.activation(out=mv[:, 1:2], in_=mv[:, 1:2],
                                     func=AF.Sqrt, bias=epsP[:], scale=-1.0)
            else:
                st = sb.tile([P, 6], F32, name=f"st_{tag}{m}", tag="st", bufs=4)
                nc.vector.bn_stats(out=st[:], in_=h_sb[m][:])
                mv = sb.tile([P, 2], F32, name=f"mv_{tag}{m}", tag="mv", bufs=4)
                nc.vector.bn_aggr(out=mv[:], in_=st[:])
                # rstd = exp(-0.5*ln(var+eps)); Ln+Exp share one ACT
                # function table (Sqrt and Exp never do), so the layer phase
                # needs zero activation-table reloads
                nc.scalar.activation(out=mv[:, 1:2], in_=mv[:, 1:2],
                                     func=AF.Ln, bias=epsP[:])
                nc.scalar.activation(out=mv[:, 1:2], in_=mv[:, 1:2],
                                     func=AF.Exp, scale=-0.5)
            at = sb.tile([P, D], BF, name=f"a_{tag}{m}", tag="af", bufs=4)
            if trivial_gb:
                nc.vector.tensor_scalar(out=at[:], in0=h_sb[m][:],
                                        scalar1=mv[:, 0:1], scalar2=mv[:, 1:2],
                                        op0=ALU.subtract, op1=ALU.mult)
            else:
                g_bc = sb.tile([P, D], F32, name=f"g_{tag}{m}", tag="gb", bufs=4)
                nc.sync.dma_start(out=g_bc[:],
                                  in_=g_dram[None, :].to_broadcast([P, D]))
                b_bc = sb.tile([P, D], F32, name=f"b_{tag}{m}", tag="gb", bufs=4)
                nc.sync.dma_start(out=b_bc[:],
                                  in_=b_dram[None, :].to_broadcast([P, D]))
                tmp = sb.tile([P, D], F32, name=f"t_{tag}{m}", tag="tmp", bufs=2)
                nc.vector.tensor_scalar(out=tmp[:], in0=h_sb[m][:],
                                        scalar1=mv[:, 0:1], scalar2=mv[:, 1:2],
                                        op0=ALU.subtract, op1=ALU.mult)
                nc.vector.tensor_tensor(out=tmp[:], in0=tmp[:], in1=g_bc[:],
                                        op=ALU.mult)
                nc.vector.tensor_tensor(out=at[:], in0=tmp[:], in1=b_bc[:],
                                        op=ALU.add)
            return at

        def ev(i, out, in_):
            if i % 2 == 0:
                nc.vector.tensor_copy(out=out, in_=in_)
            else:
                nc.scalar.copy(out, in_)

        def transpose_set(a_tiles, tag, dst_tag, alt_ev=False):
            """4x[P,D] bf16 natural -> 3x[P,T] bf16 transposed (PE + half evicts)."""
            tps, dsts = [], []
            for k in range(DT):
                tps.append(ps_tile(f"tp_{tag}{k}", T, BF))
                dsts.append(sb.tile([P, T], BF, name=f"{tag}T{k}",
                                    tag=f"{dst_tag}{k}", bufs=2))
            order = [(k, m) for m in range(TT - 1) for k in range(DT)]
            order += [(k, TT - 1) for k in range(DT)]
            done = {}
            for i, (k, m) in enumerate(order):
                nc.tensor.transpose(tps[k][:, P * m:P * (m + 1)],
                                    a_tiles[m][:, P * k:P * (k + 1)], ident[:])
                done[k] = done.get(k, 0) + 1
                if done[k] == 2:
                    ev(k if alt_ev else 0, dsts[k][:, 0:2 * P],
                       tps[k][:, 0:2 * P])
                elif done[k] == 4:
                    ev(k + 1 if alt_ev else 0, dsts[k][:, 2 * P:T],
                       tps[k][:, 2 * P:T])
            return dsts

        # ---- embedding LN (layer 0 ln1) ----
        at_cur = [ln_chain(m, io["ln1_g"][0, :], io["ln1_b"][0, :], "e")
                  for m in range(TT)]

        # ================= layers =================
        for l in range(L):
            # --- aT transposes interleaved with v (tiles 0/1 of aT feed
            # v(0), v(1) while the last LN chains finish) ---
            tps, aT = [], []
            for k in range(DT):
                tps.append(ps_tile(f"tp_a{l}{k}", T, BF))
                aT.append(sb.tile([P, T], BF, name=f"a{l}T{k}", tag=f"aT{k}",
                                  bufs=2))
            for m in range(2):
                for k in range(DT):
                    nc.tensor.transpose(tps[k][:, P * m:P * (m + 1)],
                                        at_cur[m][:, P * k:P * (k + 1)],
                                        ident[:])
            for k in range(DT):
                nc.vector.tensor_copy(out=aT[k][:, 0:2 * P],
                                      in_=tps[k][:, 0:2 * P])
            v_sb, vps = [], []
            for j in range(TT):
                vt = sb.tile([P, H, HS + 1], BF, name=f"v{l}{j}", tag=f"vv{j}",
                             bufs=1)
                v_sb.append(vt)

            def v_mms(j):
                vp = ps_tile(f"ps_v{l}{j}", D)
                vps.append(vp)
                for d in range(DT):
                    nc.tensor.matmul(vp[:], aT[d][:, P * j:P * (j + 1)],
                                     wv_t[l][:, d, :], start=(d == 0),
                                     stop=(d == DT - 1))

            def v_ev(j):
                nc.scalar.copy(v_sb[j][:, :, 0:HS],
                               vps[j][:].rearrange("p (h d) -> p h d", h=H))
                if l == 0:
                    nc.gpsimd.memset(v_sb[j][:, :, HS], 1.0)

            v_mms(0)
            v_mms(1)
            for m in range(2, TT):
                for k in range(DT):
                    nc.tensor.transpose(tps[k][:, P * m:P * (m + 1)],
                                        at_cur[m][:, P * k:P * (k + 1)],
                                        ident[:])
            for k in range(DT):
                nc.vector.tensor_copy(out=aT[k][:, 2 * P:T],
                                      in_=tps[k][:, 2 * P:T])
            v_ev(0)
            v_mms(2)
            v_ev(1)
            v_mms(3)
            v_ev(2)
            v_ev(3)

            qT = [sb.tile([P, T], BF, name=f"qT{l}{i}", tag=f"qT{i}", bufs=2)
                  for i in range(DT)]
            kT = [sb.tile([P, T], BF, name=f"kT{l}{i}", tag=f"kT{i}", bufs=2)
                  for i in range(DT)]

            def qk_mms(dq):
                qp = ps_tile(f"ps_q{l}{dq}")
                for d in range(DT):
                    nc.tensor.matmul(qp[:], wq_t[l][:, d, P * dq:P * (dq + 1)],
                                     aT[d][:], start=(d == 0), stop=(d == DT - 1))
                nc.vector.tensor_copy(out=qT[dq][:], in_=qp[:])
                kp = ps_tile(f"ps_k{l}{dq}")
                for d in range(DT):
                    nc.tensor.matmul(kp[:], wk_t[l][:, d, P * dq:P * (dq + 1)],
                                     aT[d][:], start=(d == 0), stop=(d == DT - 1))
                nc.scalar.copy(kT[dq][:], kp[:])

            if l == 0:
                load_weights(1)
            if l + 2 < L:
                load_weights(l + 2)
            if l == L - 3:
                wo = sb.tile([P, DT, VSH], BF, name="wo", tag="wo", bufs=1)
                nc.sync.dma_start(
                    out=wo[:],
                    in_=io["Wout_sh"].rearrange("(k p) n -> p k n", p=P))


            # --- attention: transposed scores, natural AV ---
            pT = {}

            def scores(j, heads=range(H)):
                n_j = T - P * j
                for h in heads:
                    r, off = (h * HS) // P, (h * HS) % P
                    ps = ps_tile(f"ps_s{l}{h}{j}", n_j)
                    nc.tensor.matmul(ps[:, 0:n_j],
                                     kT[r][off:off + HS, P * j:P * (j + 1)],
                                     qT[r][off:off + HS, P * j:T],
                                     start=True, stop=False)
                    nc.tensor.matmul(ps[:, 0:P], maskA[:], maskB[:],
                                     start=False, stop=True)
                    pt = sb.tile([P, n_j], BF, name=f"pT{l}{h}{j}", tag="pt",
                                 bufs=24)
                    nc.scalar.activation(out=pt[:, 0:n_j], in_=ps[:, 0:n_j],
                                         func=AF.Exp, scale=SCALE)
                    pT[(h, j)] = pt

            def scores23():
                # chunks j=2 (256 wide) and j=3 (128 wide) share one psum
                # and a single exp per head
                for h in range(H):
                    r, off = (h * HS) // P, (h * HS) % P
                    ps = ps_tile(f"ps_s23{l}{h}", 384)
                    nc.tensor.matmul(ps[:, 0:256],
                                     kT[r][off:off + HS, 2 * P:3 * P],
                                     qT[r][off:off + HS, 2 * P:T],
                                     start=True, stop=False)
                    nc.tensor.matmul(ps[:, 0:P], maskA[:], maskB[:],
                                     start=False, stop=True)
                    nc.tensor.matmul(ps[:, 256:384],
                                     kT[r][off:off + HS, 3 * P:T],
                                     qT[r][off:off + HS, 3 * P:T],
                                     start=True, stop=False)
                    nc.tensor.matmul(ps[:, 256:384], maskA[:], maskB[:],
                                     start=False, stop=True)
                    pt = sb.tile([P, 384], BF, name=f"pT23{l}{h}", tag="pt",
                                 bufs=24)
                    nc.scalar.activation(out=pt[:], in_=ps[:], func=AF.Exp,
                                         scale=SCALE)
                    pT[(h, 2)] = pt[:, 0:256]
                    pT[(h, 3)] = pt[:, 256:384]

            o_nat = []

            def av(m):
                avp = acc_tile(f"ps_av{l}{m}", [P, H, HS + 1])
                for h in range(H):
                    for j in range(m + 1):
                        nc.tensor.matmul(avp[:, h, :],
                                         pT[(h, j)][:, P * (m - j):P * (m - j + 1)],
                                         v_sb[j][:, h, :],
                                         start=(j == 0), stop=(j == m))
                rec = sb.tile([P, H], F32, name=f"rec{l}{m}", tag="rec", bufs=3)
                nc.vector.reciprocal(out=rec[:], in_=avp[:, :, HS])
                ot = sb.tile([P, D], BF, name=f"o{l}{m}", tag="af", bufs=4)
                nc.vector.tensor_tensor(
                    out=ot[:].rearrange("p (h d) -> p h d", h=H),
                    in0=avp[:, :, 0:HS],
                    in1=rec[:, :, None].to_broadcast([P, H, HS]), op=ALU.mult)
                o_nat.append(ot)

            qk_mms(0)
            scores(0, [0, 1])
            qk_mms(1)
            scores(1, [0, 1])
            scores(0, [2, 3])
            qk_mms(2)
            scores(1, [2, 3])
            scores(0, [4, 5])
            scores(1, [4, 5])
            av(0)
            av(1)
            scores23()
            av(2)
            av(3)

            # --- oT transposes + proj + residual + ln2 (per-tile fused) ---
            oT = transpose_set(o_nat, f"o{l}", "aT")
            bp_bc = None
            if not trivial_bias:
                bp_bc = sb.tile([P, D], F32, name=f"bp{l}", tag="gb", bufs=4)
                nc.sync.dma_start(
                    out=bp_bc[:], in_=io["bproj"][l, None, :].to_broadcast([P, D]))
            ft_cur = []
            for m in range(TT):
                pj = ps_tile(f"ps_pj{l}{m}", D)
                for k in range(DT):
                    nc.tensor.matmul(pj[:], oT[k][:, P * m:P * (m + 1)],
                                     wpj_t[l][:, k, :], start=(k == 0),
                                     stop=(k == DT - 1))
                ft_cur.append(ln_chain(m, io["ln2_g"][l, :], io["ln2_b"][l, :],
                                       f"l{l}n2", psum_add=pj, bias_bc=bp_bc,
                                       alt=False))

            # --- fT transposes + FF ---
            # FF1 first-halves of nt=0..2 fill the PE while the last LN2
            # chains finish; FF2 runs m-major so each tile's residual + next
            # LN chain starts while the remaining FF2 matmuls run.
            ftp, fT = [], []
            for k in range(DT):
                ftp.append(ps_tile(f"tp_f{l}{k}", T, BF))
                fT.append(sb.tile([P, T], BF, name=f"f{l}T{k}", tag=f"aT{k}",
                                  bufs=2))
            gt = [sb.tile([P, T], BF, name=f"g{l}{nt}", tag=f"gt{nt}", bufs=1)
                  for nt in range(NT)]
            psg = [None] * NT
            ps_h = [None] * TT

            def t_f(m):
                for k in range(DT):
                    nc.tensor.transpose(ftp[k][:, P * m:P * (m + 1)],
                                        ft_cur[m][:, P * k:P * (k + 1)],
                                        ident[:])

            def f_ev(half):
                for k in range(DT):
                    nc.vector.tensor_copy(
                        out=fT[k][:, 2 * P * half:2 * P * (half + 1)],
                        in_=ftp[k][:, 2 * P * half:2 * P * (half + 1)])

            def ff1a(nt):
                psg[nt] = ps_tile(f"ps_g{l}{nt}")
                for d in range(DT):
                    nc.tensor.matmul(psg[nt][:, 0:2 * P],
                                     w1_t[l][:, d, P * nt:P * (nt + 1)],
                                     fT[d][:, 0:2 * P], start=(d == 0),
                                     stop=(d == DT - 1))

            def ff1b(nt):
                for d in range(DT):
                    nc.tensor.matmul(psg[nt][:, 2 * P:T],
                                     w1_t[l][:, d, P * nt:P * (nt + 1)],
                                     fT[d][:, 2 * P:T], start=(d == 0),
                                     stop=(d == DT - 1))
                nc.scalar.activation(out=gt[nt][:], in_=psg[nt][:], func=AF.Relu,
                                     bias=b1_t[l][:, nt:nt + 1])

            def ff1(nt):
                psg[nt] = ps_tile(f"ps_g{l}{nt}")
                for d in range(DT):
                    nc.tensor.matmul(psg[nt][:],
                                     w1_t[l][:, d, P * nt:P * (nt + 1)],
                                     fT[d][:], start=(d == 0), stop=(d == DT - 1))
                nc.scalar.activation(out=gt[nt][:], in_=psg[nt][:], func=AF.Relu,
                                     bias=b1_t[l][:, nt:nt + 1])

            def ff2(m, nt):
                if nt == 0:
                    ps_h[m] = acc_tile(f"ps_ff{l}{m}", [P, D])
                nc.tensor.matmul(ps_h[m][:], gt[nt][:, P * m:P * (m + 1)],
                                 w2_t[l][:, nt, :], start=(nt == 0),
                                 stop=(nt == NT - 1))

            b2_bc = None
            if not trivial_bias:
                b2_bc = sb.tile([P, D], F32, name=f"b2{l}", tag="gb", bufs=4)
                nc.sync.dma_start(
                    out=b2_bc[:], in_=io["b2"][l, None, :].to_broadcast([P, D]))
            at_next = []

            def ln_next(m):
                if l + 1 < L:
                    at_next.append(ln_chain(m, io["ln1_g"][l + 1, :],
                                            io["ln1_b"][l + 1, :], f"l{l+1}n1",
                                            psum_add=ps_h[m], bias_bc=b2_bc,
                                            alt=False))
                else:
                    at_next.append(ln_chain(m, io["lnf_g"][:], io["lnf_b"][:],
                                            "lnf", psum_add=ps_h[m],
                                            bias_bc=b2_bc, alt=False))

            t_f(0)
            t_f(1)
            f_ev(0)
            ff1a(0)
            ff1a(1)
            ff1a(2)
            t_f(2)
            t_f(3)
            f_ev(1)
            ff1b(0)
            ff1b(1)
            ff1b(2)
            ff1(3)
            ff1(4)
            for nt in range(5, NT):
                ff1(nt)
                ff2(0, nt - 5)
            for nt in range(NT - 5, NT):
                ff2(0, nt)
            ln_next(0)
            for m in range(1, TT):
                for nt in range(NT):
                    ff2(m, nt)
                ln_next(m)
            at_cur = at_next

        # ---- final hidden transposed + AllGather (bf16) ----
        hfT = transpose_set(at_cur, "hf", "hf", alt_ev=True)

        if not sim_nocc:
            ag_in = dram.tile([D, T], BF, name="ag_in")
            for k in range(DT):
                nc.sync.dma_start(out=ag_in[P * k:P * (k + 1), :], in_=hfT[k][:])

        # ================= vocab-parallel head =================
        bo_bc = None
        if not trivial_bout:
            bo_bc = sb.tile([P, VSH], F32, name="bo", tag="bo", bufs=1)
            nc.sync.dma_start(
                out=bo_bc[:], in_=io["bout_sh"][None, :].to_broadcast([P, VSH]))

        if sim_nocc:
            # emulate the collective's data movement: per-(block,k) copies
            # from SBUF straight to ag_out, interleaved with the consuming
            # loads; block 0 feeds the head directly from SBUF.
            ag_out = dram.tile([N_CORES * D, T], BF, name="ag_out")

            def ag_block(rr):
                for k in range(DT):
                    nc.sync.dma_start(
                        out=ag_out[rr * D + P * k:rr * D + P * (k + 1), :],
                        in_=hfT[k][:])
        else:
            ag_out = dram.tile([N_CORES * D, T], BF, name="ag_out",
                               addr_space="Shared")
            nc.gpsimd.collective_compute(
                "AllGather", ALU.bypass,
                replica_groups=[list(range(N_CORES))],
                ins=[ag_in[:].opt()], outs=[ag_out[:].opt()])

            def ag_block(rr):
                pass

        def hb_load(b):
            hb = sb.tile([P, DT, T], BF, name=f"hf{b}", tag="hb", bufs=4)
            nc.sync.dma_start(
                out=hb[:],
                in_=ag_out[b * D:(b + 1) * D, :]
                .rearrange("(k p) n -> p k n", p=P))
            return hb

        hbs = {}
        if sim_nocc:
            hbs[0] = None            # block 0 read from SBUF hfT
            ag_block(1)
            hbs[1] = hb_load(1)
            for rr in [0] + list(range(2, N_CORES)):
                ag_block(rr)
        else:
            hbs[0] = hb_load(0)
            hbs[1] = hb_load(1)
        for b in range(N_CORES):
            hb = hbs.pop(b)
            if b + 2 < N_CORES:
                hbs[b + 2] = hb_load(b + 2)

            def lhs(d, mm_blk):
                if hb is None:
                    return hfT[d][:, mm_blk]
                return hb[:, d, mm_blk]

            for m in range(TT):
                row0 = b * T + P * m
                lo = sb.tile([P, VSH], BF, name=f"lo{b}{m}", tag="lo", bufs=5)
                for nb in range(NCH):
                    ps = ps_tile(f"ps_o{b}{m}{nb}", VCH)
                    for d in range(DT):
                        nc.tensor.matmul(ps[:], lhs(d, slice(P * m, P * (m + 1))),
                                         wo[:, d, VCH * nb:VCH * (nb + 1)],
                                         start=(d == 0), stop=(d == DT - 1))
                    sl = lo[:, VCH * nb:VCH * (nb + 1)]
                    if trivial_bout:
                        ev(nb, sl, ps[:])
                    else:
                        nc.vector.tensor_tensor(
                            out=sl, in0=ps[:],
                            in1=bo_bc[:, VCH * nb:VCH * (nb + 1)], op=ALU.add)
                    if nb == NCH // 2 - 1:
                        nc.sync.dma_start(
                            out=io["logits_sh"][row0:row0 + P, 0:VSH // 2],
                            in_=lo[:, 0:VSH // 2])
                nc.sync.dma_start(
                    out=io["logits_sh"][row0:row0 + P, VSH // 2:],
                    in_=lo[:, VSH // 2:])


_NC_CACHE = {}


def _get_nc(trivial_gb=False, trivial_bias=False, trivial_bout=False):
    key = (trivial_gb, trivial_bias, trivial_bout)
    if key not in _NC_CACHE:
        _NC_CACHE[key] = _build(*key)
    return _NC_CACHE[key]


def _build_sim():
    return _build(trivial_gb=True, trivial_bias=True, trivial_bout=True,
                  sim_nocc=True)


def kernel(**inputs):
    inp = {k: np.ascontiguousarray(np.asarray(v)) for k, v in inputs.items()}
    trivial_gb = all(
        np.all(inp[g] == 1.0) and np.all(inp[b] == 0.0)
        for g, b in [("ln1_g", "ln1_b"), ("ln2_g", "ln2_b"), ("lnf_g", "lnf_b")])
    trivial_bias = all(np.all(inp[b] == 0.0) for b in ("bproj", "b2"))
    trivial_bout = bool(np.all(inp["bout"] == 0.0))
    nc = _get_nc(trivial_gb, trivial_bias, trivial_bout)
    bf = lambda a: np.ascontiguousarray(a.astype(BF_NP))
    wq, wk, wv, wpj = bf(inp["Wq"]), bf(inp["Wk"]), bf(inp["Wv"]), bf(inp["Wproj"])
    w1, w2 = bf(inp["W1"]), bf(inp["W2"])
    in_maps = []
    for c in range(N_CORES):
        m = {
            "x": inp["x"][c].astype(np.int32),
            "tok_emb": inp["tok_emb"], "pos_emb": inp["pos_emb"],
            "ln1_g": inp["ln1_g"], "ln1_b": inp["ln1_b"],
            "Wq": wq, "Wk": wk, "Wv": wv, "Wproj": wpj, "bproj": inp["bproj"],
            "ln2_g": inp["ln2_g"], "ln2_b": inp["ln2_b"],
            "W1": w1, "b1": inp["b1"], "W2": w2, "b2": inp["b2"],
            "lnf_g": inp["lnf_g"], "lnf_b": inp["lnf_b"],
            "Wout_sh": bf(inp["Wout"][:, c * VSH:(c + 1) * VSH]),
            "bout_sh": np.ascontiguousarray(inp["bout"][c * VSH:(c + 1) * VSH]),
        }
        in_maps.append(m)
    res = run_bass_kernel_spmd(nc, in_maps, core_ids=list(range(N_CORES)))
    parts = [res.results[c]["logits_sh"].astype(np.float32).reshape(B, T, VSH)
             for c in range(N_CORES)]
    return np.concatenate(parts, axis=2)


if __name__ == "__main__":
    from concourse.timeline_sim import TimelineSim
    nc_sim = _build_sim()
    est_ns = int(TimelineSim(nc_sim, trace=False).simulate())
    print(f"sim estimate: {est_ns} ns")

